# revision 1
# baseline (speedup 1.0000x reference)
"""CLUB loss kernel for Trainium2, data-parallel over 8 NeuronCores.

Math: in the reference, mu2/lv2 (prob-model pass) are numerically identical to
mu/log_var (embedding pass) — stop_gradient only affects backward. Hence
    prob_model_loss = -mean(pos_probs)        (exactly)
    loss = embed_model_loss + prob_model_loss = -mean(neg_probs)
and the N x N x D pairwise term collapses via
    mean_j (b[j,d] - mu[i,d])^2 = msq[d] - 2*mb[d]*mu[i,d] + mu[i,d]^2
with mb = mean_j b[j,d], msq = mean_j b[j,d]^2. So
    loss = mean_i sum_d [ (msq - 2*mb*mu + mu^2) * exp(-lv) + lv ].

Sharding: rows of domain_a are split 8 x 128; each core computes the two
3-layer MLPs on its 128 rows (feature-major layout: activations stored
transposed [feature, row] so matmuls contract on partitions and biases are
per-partition), plus the b column stats, and emits a scalar partial
(sum over its rows / N). The host adds the 8 partials.
"""

import ml_dtypes
import numpy as np

import concourse.bacc as bacc
import concourse.bass as bass
import concourse.mybir as mybir
import concourse.tile as tile
from concourse.bass_utils import run_bass_kernel_spmd

N, D, H = 1024, 256, 512
NCORES = 8
ROWS = N // NCORES  # 128 rows per core
P = 128
F32 = mybir.dt.float32
BF16 = mybir.dt.bfloat16
NP_BF16 = ml_dtypes.bfloat16

_WEIGHT_SPECS = [
    ("mu_w0", (D, H)), ("mu_b0", (H,)),
    ("mu_w1", (H, H)), ("mu_b1", (H,)),
    ("mu_w2", (H, D)), ("mu_b2", (D,)),
    ("lv_w0", (D, H)), ("lv_b0", (H,)),
    ("lv_w1", (H, H)), ("lv_b1", (H,)),
    ("lv_w2", (H, D)), ("lv_b2", (D,)),
]


def _emit(nc, tc, dram, debug=False, reps=1, final_dma=True, opts=None):
    defaults = dict(b_eng='scalar', stats_late=True, interleave=True,
                    lv_relu_act=False, pack_whole=False, psum_bufs=2)
    defaults.update(opts or {})
    opts = defaults
    from contextlib import ExitStack

    AF = mybir.ActivationFunctionType
    with ExitStack() as ctx:
        pool = ctx.enter_context(tc.tile_pool(name="sbuf", bufs=1))
        psum_mm = ctx.enter_context(
            tc.tile_pool(name="psum_mm", bufs=opts["psum_bufs"], space="PSUM"))
        psum_misc = ctx.enter_context(
            tc.tile_pool(name="psum_misc", bufs=1, space="PSUM")
        )

        ones_k = pool.tile([P, 1], F32, tag="ones")
        nc.vector.memset(ones_k, 1.0)
        ones_bf = pool.tile([P, 1], BF16, tag="ones_bf")
        nc.vector.memset(ones_bf, 1.0)
        ones_row = pool.tile([1, P], BF16, tag="ones_row")
        nc.vector.memset(ones_row, 1.0)
        ident_dram = nc.inline_tensor(np.eye(P, dtype=NP_BF16), name="ident128")
        ident = pool.tile([P, P], BF16, tag="ident")
        nc.scalar.dma_start(ident, ident_dram[:, :])

        for rep in range(reps):
            # ---- a first (x0 gates both nets), then packed params ----
            a_sb = pool.tile([P, D], BF16, tag="a_sb")
            nc.sync.dma_start(a_sb, dram["a_shard"][:, :])

            # params arrive pre-packed in SBUF layout (host does the packing).
            # Small latency-critical DMAs first; weight chunks per layer so the
            # MLP starts as soon as its own layer's weights land; DMAs spread
            # over the sync + ACT sequencers (gpsimd/SWDGE carries b).
            layer_shapes = [(D, H), (H, H), (H, D)]
            bias_sb = pool.tile([1, 20 * P], BF16, tag="bias_pack")
            nc.scalar.dma_start(bias_sb, dram["bias_pack"][:, :])

            w_sb = {}  # (net, l) -> [128, K//128, M] view
            b_sb = {}  # (net, l) -> [128, M//128] view
            for ni, net in enumerate(("mu", "lv")):
                eng = nc.sync if net == "mu" else nc.scalar
                if opts['pack_whole']:
                    whole = pool.tile([P, 4096], BF16, tag=f"{net}_wpack",
                                      name=f"{net}_wpack")
                    eng.dma_start(whole, dram[f"{net}_pack"][:, :])
                woff = boff = 0
                for l, (K, M) in enumerate(layer_shapes):
                    if opts['pack_whole']:
                        wt = whole[:, woff:woff + (K // P) * M].rearrange(
                            "p (kt m) -> p kt m", kt=K // P)
                    else:
                        wt = pool.tile([P, K // P, M], BF16, tag=f"{net}_w{l}",
                                       name=f"{net}_w{l}")
                        eng.dma_start(
                            wt,
                            dram[f"{net}_pack"][:, woff:woff + (K // P) * M].rearrange(
                                "p (kt m) -> p kt m", kt=K // P
                            ),
                        )
                    w_sb[(net, l)] = wt
                    woff += (K // P) * M
                    # bf16 bias row [1, M]: applied via a K=1 rank-1 matmul
                    b_sb[(net, l)] = bias_sb[:, (ni * 10 + boff) * P:(ni * 10 + boff + M // P) * P]
                    boff += M // P

            # ---- a -> feature-major x0 via PE transpose ----
            x0 = []
            for kt in range(D // P):
                ps = psum_mm.tile([P, P], BF16, tag="mm_ps", name="tr_ps")
                nc.tensor.transpose(ps, a_sb[:, kt * P:(kt + 1) * P], ident)
                t = pool.tile([P, P], BF16, tag=f"x0_{kt}")
                nc.vector.tensor_copy(t, ps)
                x0.append(t)

            # ---- b column stats: mb2 = (-2/N) sum_j b, msq = (1/N) sum_j b^2 ----
            # NOTE: a matmul with start=True clears the whole PSUM *bank*, so each
            # accumulator needs its own bank (its own tile).
            JT = N // P  # 8
            ps_mb = [
                psum_misc.tile([P, 1], F32, tag=f"ps_mb{dh}", name=f"ps_mb{dh}")
                for dh in range(2)
            ]
            ps_ms = [
                psum_misc.tile([P, 1], F32, tag=f"ps_ms{dh}", name=f"ps_ms{dh}")
                for dh in range(2)
            ]
            # b arrives on the ACT queue after the lv weights (it is only
            # needed by the combine stage); stats matmuls are emitted after the
            # MLP so they don't preempt PE mid-chain.
            b_all = pool.tile([P, JT, D], BF16, tag="b_all")
            if opts['b_eng'] == 'split':
                b_re = dram["b_full"][:, :].rearrange("(jt p) d -> p jt d", p=P)
                nc.sync.dma_start(b_all[:, :JT // 2, :], b_re[:, :JT // 2, :])
                nc.scalar.dma_start(b_all[:, JT // 2:, :], b_re[:, JT // 2:, :])
            else:
                getattr(nc, opts['b_eng']).dma_start(
                    b_all, dram["b_full"][:, :].rearrange("(jt p) d -> p jt d", p=P)
                )
            b2_all = pool.tile([P, JT, D], BF16, tag="b2_all")
            nc.vector.tensor_mul(
                b2_all.rearrange("p j d -> p (j d)"),
                b_all.rearrange("p j d -> p (j d)"),
                b_all.rearrange("p j d -> p (j d)"),
            )

            def emit_stats():
              for jt in range(JT):
                for dh in range(D // P):
                    nc.tensor.matmul(
                        ps_mb[dh], b_all[:, jt, dh * P:(dh + 1) * P], ones_bf,
                        start=(jt == 0), stop=(jt == JT - 1), skip_group_check=True,
                    )
                    nc.tensor.matmul(
                        ps_ms[dh], b2_all[:, jt, dh * P:(dh + 1) * P], ones_bf,
                        start=(jt == 0), stop=(jt == JT - 1), skip_group_check=True,
                    )

              mb2 = pool.tile([P, 2], F32, tag="mb2", name="mb2")
              msq = pool.tile([P, 2], F32, tag="msq", name="msq")
              for dh in range(2):
                nc.scalar.mul(mb2[:, dh:dh + 1], ps_mb[dh], -2.0 / N)
                nc.scalar.mul(msq[:, dh:dh + 1], ps_ms[dh], 1.0 / N)
              return mb2, msq
            # ---- the two MLPs (feature-major: out^T[m,n] = sum_k W[k,m] x^T[k,n]).
            # Each layer accumulates into ONE psum bank: the first matmul's
            # start=True clears the bank, everything else accumulates
            # (has_written makes first element-writes overwrite). The bias is
            # added by a K=1 rank-1 matmul (bias_row^T @ ones_row), so the
            # whole layer exits through a single DVE/ACT hop.
            def run_layer(net, l, cur):
                K, M = layer_shapes[l]
                wts, brow = w_sb[(net, l)], b_sb[(net, l)]
                mts = M // P
                ps = psum_mm.tile([P, mts, P], F32, tag="mm_ps", name=f"ps_{net}{l}")
                for mt in range(mts):
                    nc.tensor.matmul(
                        ps[:, mt, :], brow[:, mt * P:(mt + 1) * P], ones_row,
                        start=(mt == 0), stop=False, skip_group_check=True,
                    )
                    for kt in range(K // P):
                        nc.tensor.matmul(
                            ps[:, mt, :], wts[:, kt, mt * P:(mt + 1) * P], cur[kt],
                            start=False, stop=(kt == K // P - 1),
                            skip_group_check=True,
                        )
                ps_flat = ps.rearrange("p a b -> p (a b)")
                if l < 2:
                    h = pool.tile([P, mts, P], BF16, tag=f"{net}_h{l}", name=f"{net}_h{l}")
                    nc.vector.tensor_scalar_max(
                        h.rearrange("p a b -> p (a b)"), ps_flat, 0.0
                    )
                elif net == "mu":
                    h = pool.tile([P, mts, P], F32, tag=f"{net}_h{l}", name=f"{net}_h{l}")
                    nc.vector.tensor_copy(h.rearrange("p a b -> p (a b)"), ps_flat)
                else:
                    h = pool.tile([P, mts, P], F32, tag=f"{net}_h{l}", name=f"{net}_h{l}")
                    nc.scalar.activation(
                        h.rearrange("p a b -> p (a b)"), ps_flat, AF.Tanh
                    )
                return [h[:, mt, :] for mt in range(mts)], h

            stats_result = None
            if not opts['stats_late']:
                stats_result = emit_stats()
            cur = {"mu": x0, "lv": x0}
            packed = {}
            if opts['interleave']:
                for l in range(3):
                    for net in ("mu", "lv"):
                        cur[net], packed[net] = run_layer(net, l, cur[net])
            else:
                for net in ("mu", "lv"):
                    for l in range(3):
                        cur[net], packed[net] = run_layer(net, l, cur[net])
            y = cur["mu"]    # pre-l2norm output, feature-major, 2 tiles [128,128]
            lv = cur["lv"]   # log_var

            if opts['stats_late']:
                mb2, msq = emit_stats()
            else:
                mb2, msq = stats_result
            iv_all = pool.tile([P, 2, P], F32, tag="iv_all")
            nc.scalar.activation(
                iv_all.rearrange("p a b -> p (a b)"),
                packed["lv"].rearrange("p a b -> p (a b)"),
                AF.Exp, scale=-1.0,
            )  # exp(-lv) over both halves in one op
            iv = [iv_all[:, kt, :] for kt in range(2)]

            if debug:
                nc.sync.dma_start(dram["dbg_mb2"][:, :], mb2)
                nc.sync.dma_start(dram["dbg_msq"][:, :], msq)
                for kt in range(2):
                    nc.sync.dma_start(dram["dbg_x0"][kt], x0[kt])
                    nc.sync.dma_start(dram["dbg_y"][kt], y[kt])
                    nc.sync.dma_start(dram["dbg_lv"][kt], lv[kt])
                    nc.sync.dma_start(dram["dbg_iv"][kt], iv[kt])

            # ---- per-row reductions over d: lhsT=data, rhs=ones -> [rows, 1] ----
            # comb regions: [0]=y^2 (-> nsq), [1]=msq*iv, [2]=(-2mb)*y*iv, [3]=y^2*iv
            # accumulator psum tiles reuse the stat banks (stats are done by now)
            acc_names = ["nsq", "sa", "sb", "sc", "sd"]
            acc_tags = ["ps_mb0", "ps_mb1", "ps_ms0", "ps_ms1", "ps_acc_sd"]
            accs = {
                n: psum_misc.tile([P, 1], F32, tag=t, name=f"acc_{n}")
                for n, t in zip(acc_names, acc_tags)
            }
            for kt in range(2):
                comb = pool.tile([P, 4, P], F32, tag=f"comb_{kt}")
                nc.vector.tensor_mul(comb[:, 0, :], y[kt], y[kt])
                nc.vector.tensor_scalar_mul(comb[:, 1, :], iv[kt], msq[:, kt:kt + 1])
                nc.vector.scalar_tensor_tensor(
                    comb[:, 2, :], y[kt], mb2[:, kt:kt + 1], iv[kt],
                    op0=mybir.AluOpType.mult, op1=mybir.AluOpType.mult,
                )
                nc.vector.tensor_mul(comb[:, 3, :], comb[:, 0, :], iv[kt])
                for r in range(4):
                    nc.tensor.matmul(accs[acc_names[r]], comb[:, r, :], ones_k,
                                     start=(kt == 0), stop=(kt == 1))
                nc.tensor.matmul(accs["sd"], lv[kt], ones_k,
                                 start=(kt == 0), stop=(kt == 1))

            # ---- finals on [128, 1] (one element per partition/row) ----
            # DVE reads the psum accumulators directly (one PSUM operand/op).
            nsq = pool.tile([P, 1], F32, tag="nsq")
            nc.vector.tensor_copy(nsq, accs["nsq"])
            # Newton rsqrt: y0 from the int32 magic, then 2 iterations.
            rinv = pool.tile([P, 1], F32, tag="rinv")
            ri = rinv.bitcast(mybir.dt.int32)
            nc.vector.tensor_scalar(
                ri, nsq.bitcast(mybir.dt.int32), 1, None,
                op0=mybir.AluOpType.logical_shift_right,
            )  # bits >> 1
            nc.vector.tensor_scalar(
                ri, ri, -1, 0x5F3759DF,
                op0=mybir.AluOpType.mult, op1=mybir.AluOpType.add,
            )  # magic - (bits >> 1)
            t1 = pool.tile([P, 1], F32, tag="t1")
            for _ in range(2):
                nc.vector.tensor_mul(t1, rinv, rinv)
                nc.vector.tensor_mul(t1, t1, nsq)
                nc.vector.tensor_scalar(
                    t1, t1, -0.5, 1.5, op0=mybir.AluOpType.mult, op1=mybir.AluOpType.add
                )
                nc.vector.tensor_mul(rinv, rinv, t1)

            row = pool.tile([P, 1], F32, tag="row")
            nc.vector.tensor_mul(row, rinv, accs["sb"])        # rinv * s_b'
            nc.vector.tensor_add(row, row, accs["sa"])
            nc.vector.tensor_add(row, row, accs["sd"])
            nc.vector.tensor_mul(t1, rinv, accs["sc"])
            nc.vector.tensor_mul(t1, t1, rinv)                 # rinv^2 * s_c
            nc.vector.tensor_add(row, row, t1)

            if debug:
                svec = pool.tile([P, 5], F32, tag="svec")
                for i, n in enumerate(acc_names):
                    nc.vector.tensor_copy(svec[:, i:i + 1], accs[n])
                nc.sync.dma_start(dram["dbg_red"][:, :], svec[:, 0:4])
                nc.sync.dma_start(dram["dbg_sd"][:, :], svec[:, 4:5])

            ps_total = psum_misc.tile([1, 1], F32, tag="ps_mb0", name="ps_total")
            nc.tensor.matmul(ps_total, row, ones_k, start=True, stop=True)
            final = pool.tile([1, 1], F32, tag="final")
            nc.vector.tensor_copy(final, ps_total)
            if final_dma and rep == reps - 1:
                nc.sync.dma_start(dram["partial"][:, :], final)


_NC_CACHE = {}
_OPTS = {}


def _build(reps=1):
    if reps in _NC_CACHE:
        return _NC_CACHE[reps]
    nc = bacc.Bacc("TRN2", target_bir_lowering=False, debug=False)
    dram = {
        "a_shard": nc.dram_tensor("a_shard", [ROWS, D], BF16, kind="ExternalInput"),
        "b_full": nc.dram_tensor("b_full", [N, D], BF16, kind="ExternalInput"),
        "mu_pack": nc.dram_tensor("mu_pack", [P, 4096], BF16, kind="ExternalInput"),
        "lv_pack": nc.dram_tensor("lv_pack", [P, 4096], BF16, kind="ExternalInput"),
        "bias_pack": nc.dram_tensor("bias_pack", [1, 20 * P], BF16, kind="ExternalInput"),
        "partial": nc.dram_tensor("partial", [1, 1], F32, kind="ExternalOutput"),
    }
    with tile.TileContext(nc) as tc:
        _emit(nc, tc, dram, reps=reps, opts=_OPTS)
    nc.compile()
    _NC_CACHE[reps] = nc
    return nc


def _pack_params(inputs):
    """Pack weights/biases into the exact SBUF layouts the kernel DMAs."""
    packs = {}
    for net in ("mu", "lv"):
        cols = []
        for l in range(3):
            w = np.asarray(inputs[f"{net}_w{l}"], np.float32)
            K, M = w.shape
            # [K, M] -> [128, (K//128)*M], partition-major k-tiles
            cols.append(w.reshape(K // P, P, M).transpose(1, 0, 2).reshape(P, -1))
        packs[f"{net}_pack"] = np.ascontiguousarray(
            np.concatenate(cols, axis=1), dtype=NP_BF16
        )
    bcols = []
    for net in ("mu", "lv"):
        for l in range(3):
            bcols.append(np.asarray(inputs[f"{net}_b{l}"], np.float32).ravel())
    packs["bias_pack"] = np.ascontiguousarray(
        np.concatenate(bcols).reshape(1, 20 * P), dtype=NP_BF16
    )
    return packs


def kernel_with_results(**inputs):
    import os
    try:
        import antenv.axon_hooks  # noqa: F401
    except ImportError:
        # run_bass_kernel_spmd's trace path needs this module; without it a
        # stray BASS_TRACE=1 in the environment would crash the run.
        os.environ.setdefault("BASS_NEVER_TRACE", "1")
    nc = _build()
    a = np.ascontiguousarray(np.asarray(inputs["domain_a"], np.float32), dtype=NP_BF16)
    b = np.ascontiguousarray(np.asarray(inputs["domain_b"], np.float32), dtype=NP_BF16)
    base = _pack_params(inputs)
    base["b_full"] = b
    in_maps = [
        dict(base, a_shard=np.ascontiguousarray(a[c * ROWS:(c + 1) * ROWS]))
        for c in range(NCORES)
    ]
    res = run_bass_kernel_spmd(nc, in_maps, core_ids=list(range(NCORES)))
    total = np.float64(0.0)
    for r in res.results:
        total += np.float64(r["partial"][0, 0])
    total /= N
    return np.asarray(total, dtype=np.float32).reshape(()), res


def kernel(**inputs):
    out, _ = kernel_with_results(**inputs)
    return out



# revision 5
# speedup vs baseline: 1.2256x; 1.2256x over previous
"""CLUB loss kernel for Trainium2, data-parallel over 8 NeuronCores.

Math: mu2/lv2 (prob-model pass) are numerically identical to mu/log_var
(stop_gradient only affects backward), so
    loss = embed_model_loss + prob_model_loss = -mean(neg_probs)
and with mb = mean_j b[j,d], msq = mean_j b[j,d]^2 the N x N x D pairwise
term collapses:
    loss*N = sum_i sum_d [ (msq - 2*mb*mu + mu^2) * iv + lv ],  iv = exp(-lv).

Decomposition per core c (128 rows of domain_a, 128 rows of domain_b):
    scalar_c  = sum_i [ sum_d lv + rinv_i^2 * sum_d y^2*iv ]   (y = pre-norm mu)
    SB_c[d]   = sum_i iv[i,d]
    SC_c[d]   = sum_i mu[i,d]*iv[i,d]
    pmb_c[d]  = sum_{j in shard} b[j,d],  pmsq_c[d] = sum_j b[j,d]^2
Host combine (the cross-core all-reduce, done on the gathered partials):
    mb = sum_c pmb_c / N, msq = sum_c pmsq_c / N
    loss = (sum_c scalar_c + msq . sum_c SB_c - 2*mb . sum_c SC_c) / N

On-device: both 3-layer MLPs run in fp8-e4m3 with DoubleRow matmuls
(weights stationary, feature-major) for L0/L1; L2 flips to row-major
(activations stationary) so the tail reduces over d via accum_out and over
i via 1-column matmuls. All inputs arrive in one u8 mega-pack split into a
few large DMAs (per-DMA fixed costs dominate small transfers); the bias row
rides the software DGE so it skips the HWDGE queue. Biases are added into
PSUM via rank-1 bf16 matmuls interleaved with each layer's DoubleRow group.
The mu-net evacuates through DVE and the lv-net through ACT (single act
table set: Relu/Tanh/Exp), and rsqrt is a Newton iteration on DVE.
"""

import ml_dtypes
import numpy as np

import concourse.bacc as bacc
import concourse.bass as bass
import concourse.mybir as mybir
import concourse.tile as tile
from concourse.bass_utils import run_bass_kernel_spmd

N, D, H = 1024, 256, 512
NCORES = 8
ROWS = N // NCORES  # 128 rows per core
P = 128
F32 = mybir.dt.float32
I32 = mybir.dt.int32
BF16 = mybir.dt.bfloat16
F8 = mybir.dt.float8e4
U8 = mybir.dt.uint8
NP_BF16 = ml_dtypes.bfloat16
NP_F8 = ml_dtypes.float8_e4m3

S_A = 32.0     # fp8 scale on domain_a
S_W = 2048.0   # fp8 scale on all weights
S_H = 32.0     # fp8 scale on hidden activations

# u8 column offsets in the per-core mega-pack
A_OFF = 0                 # a: fp8 [128, 2, 128] (transposed, k-tiled)
W0_OFF = 256              # L0 weights: per net 4 mt-tiles x 256 cols
W1_OFF = W0_OFF + 2048    # L1: per net 8 (mt,g)-tiles x 256 cols
W2_OFF = W1_OFF + 4096    # L2: per net 2 g-tiles x 512 cols (row-major rhs)
B_OFF = W2_OFF + 2048     # b shard: bf16 [128, 256]
PACK_COLS = B_OFF + 512

# DMA chunk boundaries (u8 cols): a+L0 | mu-L1 | lv-L1 | L2+b
CHUNKS = [(0, W1_OFF), (W1_OFF, W1_OFF + 2048), (W1_OFF + 2048, W2_OFF),
          (W2_OFF, PACK_COLS)]

# bias row: bf16 [1, 2560]; per (net, layer) offsets in bf16 units
BIAS_OFF = {("mu", 0): 0, ("mu", 1): 512, ("mu", 2): 1024,
            ("lv", 0): 1280, ("lv", 1): 1792, ("lv", 2): 2304}


def _emit(nc, tc, dram, opts=None):
    defaults = dict(chunks=CHUNKS, warmup=0, gaps=(), newton=2, psum_out=False)
    defaults.update(opts or {})
    opts = defaults
    from contextlib import ExitStack

    AF = mybir.ActivationFunctionType
    DR = mybir.MatmulPerfMode.DoubleRow
    MUL = mybir.AluOpType.mult
    ADD = mybir.AluOpType.add
    MAX = mybir.AluOpType.max
    SHR = mybir.AluOpType.logical_shift_right

    with ExitStack() as ctx:
        pool = ctx.enter_context(tc.tile_pool(name="sbuf", bufs=1))
        psum = ctx.enter_context(tc.tile_pool(name="psum", bufs=1, space="PSUM"))

        # ---- constants ----
        ones_row = pool.tile([1, P], BF16, tag="ones_row")
        nc.vector.memset(ones_row, 1.0)
        ones_col = pool.tile([P, 1], BF16, tag="ones_col")
        nc.vector.memset(ones_col, 1.0)
        ones_sq = pool.tile([P, P], F32, tag="ones_sq")
        nc.vector.memset(ones_sq, 1.0)

        # ---- input DMAs: pack chunks on sync/HWDGE, bias row on SWDGE ----
        chunk_sb = []
        for (s, e) in opts["chunks"]:
            t = pool.tile([P, e - s], U8, tag=f"chunk_{s}", name=f"chunk_{s}")
            nc.sync.dma_start(t, dram["pack"][:, s:e])
            chunk_sb.append((s, e, t))
        bias_u8 = pool.tile([1, 5120], U8, tag="bias")
        nc.gpsimd.dma_start(bias_u8, dram["bias"][:, :])
        bias_bf = bias_u8[:, :].bitcast(BF16)  # [1, 2560]

        def view(off, ncols, dtype, kt=None):
            for (s, e, t) in chunk_sb:
                if off >= s and off + ncols <= e:
                    v = t[:, off - s:off - s + ncols].bitcast(dtype)
                    if kt is not None:
                        v = v.rearrange("p (kt m) -> p kt m", kt=kt)
                    return v
            raise AssertionError(f"cols [{off},{off + ncols}) straddle chunks")

        a_v = view(A_OFF, 256, F8, kt=2)            # [128, 2, 128]
        w0 = {net: [view(W0_OFF + ni * 1024 + mt * 256, 256, F8, kt=2)
                    for mt in range(4)]
              for ni, net in enumerate(("mu", "lv"))}
        w1 = {net: [[view(W1_OFF + ni * 2048 + (mt * 2 + g) * 256, 256, F8, kt=2)
                     for g in range(2)] for mt in range(4)]
              for ni, net in enumerate(("mu", "lv"))}
        w2 = {net: [view(W2_OFF + ni * 1024 + g * 512, 512, F8, kt=2)
                    for g in range(2)]
              for ni, net in enumerate(("mu", "lv"))}
        b_v = view(B_OFF, 512, BF16)                # [128, 256] row-major shard

        def brow(net, l, mt=None, m=P):
            off = BIAS_OFF[(net, l)]
            if mt is not None:
                off += mt * P
            return bias_bf[:, off:off + m]

        # ---- psum banks ----
        ps0 = {net: psum.tile([P, 4, P], F32, tag=f"ps0_{net}", name=f"ps0_{net}")
               for net in ("mu", "lv")}
        ps1 = {net: psum.tile([P, 4, P], F32, tag=f"ps1_{net}", name=f"ps1_{net}")
               for net in ("mu", "lv")}
        ps2 = {net: psum.tile([P, 2 * P], F32, tag=f"ps2_{net}", name=f"ps2_{net}")
               for net in ("mu", "lv")}
        ps_out = psum.tile([P, 9], F32, tag="ps_out", name="ps_out")

        mm = nc.tensor.matmul

        # ---- optional PE warm-up: keep PE busy so the p-state ramps ----
        if opts["warmup"]:
            ps_w = psum.tile([P, P], F32, tag="ps_warm", name="ps_warm")

            def dummies(k):
                for _ in range(k):
                    mm(ps_w, ones_row, ones_row, start=True, stop=True,
                       skip_group_check=True)
        else:
            def dummies(k):
                pass
        gaps = dict(opts["gaps"])
        dummies(opts["warmup"])

        # ---- MLP: per net-layer [bias rank-1 x mt, DoubleRow group] ----
        def bias01(net, l, ps):
            for mt in range(4):
                mm(ps[net][:, mt, :], brow(net, l, mt), ones_row,
                   start=(mt == 0), stop=False, skip_group_check=True)

        h1 = {net: pool.tile([P, 4, P], F8, tag=f"h1_{net}", name=f"h1_{net}")
              for net in ("mu", "lv")}
        h2 = {net: pool.tile([P, 4, P], F8, tag=f"h2_{net}", name=f"h2_{net}")
              for net in ("mu", "lv")}
        EV0 = S_H / (S_A * S_W)
        EV1 = 1.0 / S_W
        EV2 = 1.0 / (S_H * S_W)

        def evac_relu(net, l, src_ps, dst, scale):
            """Evacuate [128, 4, 128] psum -> fp8 relu, in halves."""
            eng = nc.vector if net == "mu" else nc.scalar
            for half in range(2):
                s = src_ps[net][:, 2 * half:2 * half + 2, :].rearrange(
                    "p a b -> p (a b)")
                d = dst[net][:, 2 * half:2 * half + 2, :].rearrange(
                    "p a b -> p (a b)")
                if net == "mu":
                    eng.tensor_scalar(d, s, scale, 0.0, op0=MUL, op1=MAX)
                else:
                    eng.activation(d, s, AF.Relu, scale=scale)

        # L0
        bias01("mu", 0, ps0)
        for mt in range(4):
            mm(ps0["mu"][:, mt, :], w0["mu"][mt], a_v,
               start=False, stop=(mt == 3), perf_mode=DR, skip_group_check=True)
        dummies(gaps.get("l0", 0))
        bias01("lv", 0, ps0)
        for mt in range(4):
            mm(ps0["lv"][:, mt, :], w0["lv"][mt], a_v,
               start=False, stop=(mt == 3), perf_mode=DR, skip_group_check=True)
        evac_relu("mu", 0, ps0, h1, EV0)
        evac_relu("lv", 0, ps0, h1, EV0)

        # L1 (g-outer so the first half of h1 unblocks the first 4 matmuls)
        dummies(gaps.get("l1", 0))
        for net in ("mu", "lv"):
            bias01(net, 1, ps1)
            for g in range(2):
                for mt in range(4):
                    mm(ps1[net][:, mt, :], w1[net][mt][g],
                       h1[net][:, 2 * g:2 * g + 2, :],
                       start=False, stop=(mt == 3 and g == 1), perf_mode=DR,
                       skip_group_check=True)
            evac_relu(net, 1, ps1, h2, EV1)

        # L2 row-major: psum[i, d] += sum_k h2[k, i] * W2[k, d]
        dummies(gaps.get("l2", 0))
        for net in ("mu", "lv"):
            mm(ps2[net], ones_row, brow(net, 2, m=2 * P),
               start=True, stop=False, skip_group_check=True)
        for g in range(2):
            mm(ps2["mu"], h2["mu"][:, 2 * g:2 * g + 2, :], w2["mu"][g],
               start=False, stop=(g == 1), perf_mode=DR, skip_group_check=True)

        # stats on own b shard -> ps_out cols 4..7 (pmb0 opens the bank)
        b2 = pool.tile([P, 2 * P], BF16, tag="b2")
        nc.gpsimd.tensor_mul(b2, b_v, b_v)
        for dh in range(2):
            mm(ps_out[:, 4 + dh:5 + dh], b_v[:, dh * P:(dh + 1) * P], ones_col,
               start=(dh == 0), stop=False, skip_group_check=True)
        for dh in range(2):
            mm(ps_out[:, 6 + dh:7 + dh], b2[:, dh * P:(dh + 1) * P], ones_col,
               start=False, stop=False, skip_group_check=True)

        for g in range(2):
            mm(ps2["lv"], h2["lv"][:, 2 * g:2 * g + 2, :], w2["lv"][g],
               start=False, stop=(g == 1), perf_mode=DR, skip_group_check=True)

        # ---- tail (row-major [i, d]) ----
        y = pool.tile([P, 2 * P], BF16, tag="y")
        nc.vector.tensor_scalar_mul(y, ps2["mu"], EV2)
        y2 = pool.tile([P, 2 * P], BF16, tag="y2")
        nsq = pool.tile([P, 1], F32, tag="nsq")
        nc.vector.scalar_tensor_tensor(y2, y, 1.0, y, op0=MUL, op1=MUL,
                                       accum_out=nsq)

        lv_sb = pool.tile([P, 2 * P], F32, tag="lv_sb")
        sd = pool.tile([P, 1], F32, tag="sd")
        nc.scalar.activation(lv_sb, ps2["lv"], AF.Tanh, scale=EV2, accum_out=sd)
        iv = pool.tile([P, 2 * P], BF16, tag="iv")
        nc.scalar.activation(iv, lv_sb, AF.Exp, scale=-1.0)

        # Newton rsqrt on DVE: seed from the int32 magic, then iterations
        rinv = pool.tile([P, 1], F32, tag="rinv")
        ri = rinv.bitcast(I32)
        nc.vector.tensor_scalar(ri, nsq.bitcast(I32), 1, None, op0=SHR)
        nc.vector.tensor_scalar(ri, ri, -1, 0x5F3759DF, op0=MUL, op1=ADD)
        nh = pool.tile([P, 1], F32, tag="nh")
        nc.vector.tensor_scalar_mul(nh, nsq, -0.5)
        t1 = pool.tile([P, 1], F32, tag="t1")
        for _ in range(opts["newton"]):
            nc.vector.tensor_mul(t1, rinv, rinv)
            nc.vector.tensor_scalar(t1, t1, nh, 1.5, op0=MUL, op1=ADD)
            nc.vector.tensor_mul(rinv, rinv, t1)

        muiv = pool.tile([P, 2 * P], BF16, tag="muiv")
        nc.vector.scalar_tensor_tensor(muiv, y, rinv, iv, op0=MUL, op1=MUL)
        y2iv = pool.tile([P, 2 * P], BF16, tag="y2iv")
        sc = pool.tile([P, 1], F32, tag="sc")
        nc.vector.scalar_tensor_tensor(y2iv, y2, 1.0, iv, op0=MUL, op1=MUL,
                                       accum_out=sc)
        nc.vector.tensor_scalar(t1, sc, rinv, None, op0=MUL)
        row = pool.tile([P, 1], F32, tag="row")
        nc.vector.scalar_tensor_tensor(row, t1, rinv, sd, op0=MUL, op1=ADD)

        # ---- SB / SC / scalar -> ps_out cols 0..3, 8 ----
        for dh in range(2):
            mm(ps_out[:, 0 + dh:1 + dh], iv[:, dh * P:(dh + 1) * P], ones_col,
               start=False, stop=False, skip_group_check=True)
        for dh in range(2):
            mm(ps_out[:, 2 + dh:3 + dh], muiv[:, dh * P:(dh + 1) * P], ones_col,
               start=False, stop=False, skip_group_check=True)
        # broadcast the scalar total to every partition of col 8 so the bank
        # is fully written (lhsT = all-ones [128, 128])
        mm(ps_out[:, 8:9], ones_sq, row,
           start=False, stop=True, skip_group_check=True)

        if opts["psum_out"]:
            nc.sync.dma_start(dram["out"][:, :], ps_out)
        else:
            out_sb = pool.tile([P, 9], F32, tag="out_sb")
            nc.vector.tensor_copy(out_sb, ps_out)
            nc.sync.dma_start(dram["out"][:, :], out_sb)


_NC_CACHE = {}
_OPTS = {}


def _build(reps=1):
    key = ("v2", reps, tuple(sorted(_OPTS.items())))
    if key in _NC_CACHE:
        return _NC_CACHE[key]
    nc = bacc.Bacc("TRN2", target_bir_lowering=False, debug=False)
    dram = {
        "pack": nc.dram_tensor("pack", [P, PACK_COLS], U8, kind="ExternalInput"),
        "bias": nc.dram_tensor("bias", [1, 5120], U8, kind="ExternalInput"),
        "out": nc.dram_tensor("out", [P, 9], F32, kind="ExternalOutput"),
    }
    with tile.TileContext(nc) as tc:
        _emit(nc, tc, dram, opts=_OPTS)
    nc.compile()
    _NC_CACHE[key] = nc
    return nc


def _pack_host(inputs):
    """Build the weight/bias packs (shared across cores) and per-core packs."""
    f32 = np.float32

    def fp8(x):
        return np.asarray(x, f32).astype(NP_F8)

    wcols = np.empty((P, B_OFF - W0_OFF), np.uint8)  # weights only
    col = 0
    for net in ("mu", "lv"):
        w = fp8(np.asarray(inputs[f"{net}_w0"], f32) * S_W)  # [256, 512]
        t = w.reshape(2, P, 4, P).transpose(2, 1, 0, 3)       # [mt, p, kt, m]
        wcols[:, col:col + 1024] = t.transpose(1, 0, 2, 3).reshape(P, 1024).view(np.uint8)
        col += 1024
    for net in ("mu", "lv"):
        w = fp8(np.asarray(inputs[f"{net}_w1"], f32) * S_W)  # [512, 512]
        # tile (mt, g): [p, kt, m] = w[(2g+kt)*128+p, mt*128+m]
        t = w.reshape(2, 2, P, 4, P)                          # [g, kt, p, mt, m]
        t = t.transpose(3, 0, 2, 1, 4)                        # [mt, g, p, kt, m]
        wcols[:, col:col + 2048] = t.transpose(2, 0, 1, 3, 4).reshape(P, 2048).view(np.uint8)
        col += 2048
    for net in ("mu", "lv"):
        w = fp8(np.asarray(inputs[f"{net}_w2"], f32) * S_W)  # [512, 256]
        t = w.reshape(2, 2, P, 2 * P)                         # [g, kt, p, m]
        t = t.transpose(0, 2, 1, 3)                           # [g, p, kt, m]
        wcols[:, col:col + 1024] = t.transpose(1, 0, 2, 3).reshape(P, 1024).view(np.uint8)
        col += 1024
    assert col == wcols.shape[1]

    brow = np.empty((1, 2560), NP_BF16)
    for net in ("mu", "lv"):
        for l, s in ((0, S_A * S_W), (1, S_H * S_W), (2, S_H * S_W)):
            off = BIAS_OFF[(net, l)]
            bb = np.asarray(inputs[f"{net}_b{l}"], f32) * s
            brow[0, off:off + bb.size] = bb.astype(NP_BF16)
    bias_u8 = np.ascontiguousarray(brow.view(np.uint8))       # [1, 5120]

    a = np.asarray(inputs["domain_a"], f32)
    b = np.asarray(inputs["domain_b"], f32)
    packs = []
    for c in range(NCORES):
        pk = np.empty((P, PACK_COLS), np.uint8)
        ash = fp8(a[c * ROWS:(c + 1) * ROWS] * S_A)           # [128 rows, 256 d]
        # a_pack[p, kt, n] = a[n, kt*128+p]
        at = ash.T.reshape(2, P, P).transpose(1, 0, 2).reshape(P, 256)
        pk[:, A_OFF:A_OFF + 256] = np.ascontiguousarray(at).view(np.uint8)
        pk[:, W0_OFF:B_OFF] = wcols
        bsh = np.asarray(b[c * ROWS:(c + 1) * ROWS], f32).astype(NP_BF16)
        pk[:, B_OFF:] = np.ascontiguousarray(bsh).view(np.uint8)
        packs.append(pk)
    return packs, bias_u8


def kernel_with_results(**inputs):
    import os
    try:
        import antenv.axon_hooks  # noqa: F401
    except ImportError:
        # run_bass_kernel_spmd's trace path needs this module; without it a
        # stray BASS_TRACE=1 in the environment would crash the run.
        os.environ.setdefault("BASS_NEVER_TRACE", "1")
    nc = _build()
    packs, bias_u8 = _pack_host(inputs)
    in_maps = [dict(pack=packs[c], bias=bias_u8) for c in range(NCORES)]
    res = run_bass_kernel_spmd(nc, in_maps, core_ids=list(range(NCORES)))

    scal = np.float64(0.0)
    SB = np.zeros(D, np.float64)
    SC = np.zeros(D, np.float64)
    pmb = np.zeros(D, np.float64)
    pmsq = np.zeros(D, np.float64)
    for r in res.results:
        o = np.asarray(r["out"], np.float64)
        SB += np.concatenate([o[:, 0], o[:, 1]])
        SC += np.concatenate([o[:, 2], o[:, 3]])
        pmb += np.concatenate([o[:, 4], o[:, 5]])
        pmsq += np.concatenate([o[:, 6], o[:, 7]])
        scal += o[0, 8]
    mb = pmb / N
    msq = pmsq / N
    loss = (scal + msq @ SB - 2.0 * (mb @ SC)) / N
    return np.asarray(loss, dtype=np.float32).reshape(()), res


def kernel(**inputs):
    out, _ = kernel_with_results(**inputs)
    return out


# revision 18
# speedup vs baseline: 1.4224x; 1.1606x over previous
"""CLUB loss kernel for Trainium2, data-parallel over 8 NeuronCores.

Math: mu2/lv2 (prob-model pass) are numerically identical to mu/log_var
(stop_gradient only affects backward), so
    loss = embed_model_loss + prob_model_loss = -mean(neg_probs)
and with mb = mean_j b[j,d], msq = mean_j b[j,d]^2 the N x N x D pairwise
term collapses:
    loss*N = sum_i sum_d [ (msq - 2*mb*mu + mu^2) * iv + lv ],  iv = exp(-lv).

Decomposition per core c (128 rows of domain_a, 128 rows of domain_b):
    scalar_c  = sum_i [ sum_d lv + rinv_i^2 * sum_d y^2*iv ]   (y = pre-norm mu)
    SB_c[d]   = sum_i iv[i,d]
    SC_c[d]   = sum_i mu[i,d]*iv[i,d]
    pmb_c[d]  = sum_{j in shard} b[j,d],  pmsq_c[d] = sum_j b[j,d]^2
Host combine (the cross-core all-reduce, done on the gathered partials):
    mb = sum_c pmb_c / N, msq = sum_c pmsq_c / N
    loss = (sum_c scalar_c + msq . sum_c SB_c - 2*mb . sum_c SC_c) / N

On-device: both 3-layer MLPs run in fp8-e4m3 with DoubleRow matmuls
(weights stationary, feature-major) for L0/L1; L2 flips to row-major
(activations stationary) so the tail reduces over d via accum_out and over
i via 1-column matmuls. All inputs arrive in one u8 mega-pack split into a
few large DMAs (per-DMA fixed costs dominate small transfers); the bias row
rides the software DGE so it skips the HWDGE queue. Biases are added into
PSUM via rank-1 bf16 matmuls interleaved with each layer's DoubleRow group.
The mu-net evacuates through DVE and the lv-net through ACT (single act
table set: Relu/Tanh/Exp), and rsqrt is a Newton iteration on DVE.
"""

import ml_dtypes
import numpy as np

import concourse.bacc as bacc
import concourse.bass as bass
import concourse.mybir as mybir
import concourse.tile as tile
from concourse.bass_utils import run_bass_kernel_spmd

N, D, H = 1024, 256, 512
NCORES = 8
ROWS = N // NCORES  # 128 rows per core
P = 128
F32 = mybir.dt.float32
I32 = mybir.dt.int32
BF16 = mybir.dt.bfloat16
F8 = mybir.dt.float8e4
U8 = mybir.dt.uint8
NP_BF16 = ml_dtypes.bfloat16
NP_F8 = ml_dtypes.float8_e4m3

S_A = 32.0     # fp8 scale on domain_a
S_W = 2048.0   # fp8 scale on all weights
S_H = 32.0     # fp8 scale on hidden activations

# u8 column offsets in the per-core mega-pack
A_OFF = 0                 # a: fp8 [128, 2, 128] (transposed, k-tiled)
W0_OFF = 256              # L0 weights: per net 4 mt-tiles x 256 cols
B_OFF = W0_OFF + 2048     # b shard: bf16 [128, 256] (early: stats feed PE)
W1_OFF = B_OFF + 512      # L1: per net 8 (mt,g)-tiles x 256 cols
W2_OFF = W1_OFF + 4096    # L2: per net 2 g-tiles x 512 cols (row-major rhs)
PACK_COLS = W2_OFF + 2048

# DMA chunk boundaries (u8 cols): a+L0+b | mu-L1 | lv-L1 | mu-L2 | lv-L2
CHUNKS = [(0, W1_OFF), (W1_OFF, W1_OFF + 2048), (W1_OFF + 2048, W2_OFF),
          (W2_OFF, W2_OFF + 1024), (W2_OFF + 1024, PACK_COLS)]

# bias row: bf16 [1, 2560]; per (net, layer) offsets in bf16 units
BIAS_OFF = {("mu", 0): 0, ("mu", 1): 512, ("mu", 2): 1024,
            ("lv", 0): 1280, ("lv", 1): 1792, ("lv", 2): 2304}


def _emit(nc, tc, dram, opts=None):
    defaults = dict(chunks=CHUNKS, warmup=0, gaps=(), newton=2, psum_out=False)
    defaults.update(opts or {})
    opts = defaults
    from contextlib import ExitStack

    AF = mybir.ActivationFunctionType
    DR = mybir.MatmulPerfMode.DoubleRow
    MUL = mybir.AluOpType.mult
    ADD = mybir.AluOpType.add
    MAX = mybir.AluOpType.max
    SHR = mybir.AluOpType.logical_shift_right

    with ExitStack() as ctx:
        pool = ctx.enter_context(tc.tile_pool(name="sbuf", bufs=1))
        psum = ctx.enter_context(tc.tile_pool(name="psum", bufs=1, space="PSUM"))

        # ---- constants ----
        ones_row = pool.tile([1, P], BF16, tag="ones_row")
        nc.vector.memset(ones_row, 1.0)
        ones_col = pool.tile([P, 1], BF16, tag="ones_col")
        nc.vector.memset(ones_col, 1.0)
        ones_sq = pool.tile([P, P], F32, tag="ones_sq")
        nc.vector.memset(ones_sq, 1.0)

        # ---- input DMAs: pack chunks on sync/HWDGE, bias row on SWDGE ----
        chunk_sb = []
        for (s, e) in opts["chunks"]:
            t = pool.tile([P, e - s], U8, tag=f"chunk_{s}", name=f"chunk_{s}")
            nc.sync.dma_start(t, dram["pack"][:, s:e])
            chunk_sb.append((s, e, t))
        bias_u8 = pool.tile([1, 5120], U8, tag="bias")
        nc.gpsimd.dma_start(bias_u8, dram["bias"][:, :])
        bias_bf = bias_u8[:, :].bitcast(BF16)  # [1, 2560]

        def view(off, ncols, dtype, kt=None):
            for (s, e, t) in chunk_sb:
                if off >= s and off + ncols <= e:
                    v = t[:, off - s:off - s + ncols].bitcast(dtype)
                    if kt is not None:
                        v = v.rearrange("p (kt m) -> p kt m", kt=kt)
                    return v
            raise AssertionError(f"cols [{off},{off + ncols}) straddle chunks")

        a_v = view(A_OFF, 256, F8, kt=2)            # [128, 2, 128]
        w0 = {net: [view(W0_OFF + ni * 1024 + mt * 256, 256, F8, kt=2)
                    for mt in range(4)]
              for ni, net in enumerate(("mu", "lv"))}
        w1 = {net: [[view(W1_OFF + ni * 2048 + (mt * 2 + g) * 256, 256, F8, kt=2)
                     for g in range(2)] for mt in range(4)]
              for ni, net in enumerate(("mu", "lv"))}
        w2 = {net: [view(W2_OFF + ni * 1024 + g * 512, 512, F8, kt=2)
                    for g in range(2)]
              for ni, net in enumerate(("mu", "lv"))}
        b_v = view(B_OFF, 512, BF16)                # [128, 256] row-major shard

        def brow(net, l, mt=None, m=P):
            off = BIAS_OFF[(net, l)]
            if mt is not None:
                off += mt * P
            return bias_bf[:, off:off + m]

        # ---- psum banks ----
        ps0 = {net: psum.tile([P, 4, P], F32, tag=f"ps0_{net}", name=f"ps0_{net}")
               for net in ("mu", "lv")}
        ps1 = {net: psum.tile([P, 4, P], F32, tag=f"ps1_{net}", name=f"ps1_{net}")
               for net in ("mu", "lv")}
        ps2 = {net: psum.tile([P, 2 * P], F32, tag=f"ps2_{net}", name=f"ps2_{net}")
               for net in ("mu", "lv")}
        ps_out = psum.tile([P, 10], F32, tag="ps_out", name="ps_out")

        mm = nc.tensor.matmul

        # Clear the ps_out bank early via a junk matmul into col 9 (start=True
        # zeroes the whole bank), so every real ps_out matmul can be
        # start=False and the scheduler is free to order them by readiness.
        mm(ps_out[:, 9:10], ones_sq, ones_sq[:, 0:1],
           start=True, stop=False, skip_group_check=True)

        # ---- optional PE warm-up: keep PE busy so the p-state ramps ----
        if opts["warmup"]:
            ps_w = psum.tile([P, P], F32, tag="ps_warm", name="ps_warm")

            def dummies(k):
                for _ in range(k):
                    mm(ps_w, ones_row, ones_row, start=True, stop=True,
                       skip_group_check=True)
        else:
            def dummies(k):
                pass
        gaps = dict(opts["gaps"])
        dummies(opts["warmup"])

        # ---- MLP ----
        h1 = {net: pool.tile([P, 4, P], F8, tag=f"h1_{net}", name=f"h1_{net}")
              for net in ("mu", "lv")}
        h2 = {net: pool.tile([P, 4, P], F8, tag=f"h2_{net}", name=f"h2_{net}")
              for net in ("mu", "lv")}
        EV0 = S_H / (S_A * S_W)
        EV1 = 1.0 / S_W
        EV2 = 1.0 / (S_H * S_W)

        def bias01(net, l, ps, start):
            for mt in range(4):
                mm(ps[net][:, mt, :], brow(net, l, mt), ones_row,
                   start=(start and mt == 0), stop=False, skip_group_check=True)

        def evac_relu(net, l, src_ps, dst, scale, engs):
            """Evacuate [128, 4, 128] psum -> fp8 relu, halves on 2 engines."""
            for half, eng in enumerate(engs):
                s = src_ps[net][:, 2 * half:2 * half + 2, :].rearrange(
                    "p a b -> p (a b)")
                d = dst[net][:, 2 * half:2 * half + 2, :].rearrange(
                    "p a b -> p (a b)")
                if eng is nc.scalar:
                    eng.activation(d, s, AF.Relu, scale=scale)
                else:
                    eng.tensor_scalar(d, s, scale, 0.0, op0=MUL, op1=MAX)

        # L0: DR first (opens the bank; gated only on chunk A), bias after
        # (gated on the SWDGE bias row which lands slightly later)
        for net in ("mu", "lv"):
            for mt in range(4):
                mm(ps0[net][:, mt, :], w0[net][mt], a_v,
                   start=(mt == 0), stop=False, perf_mode=DR,
                   skip_group_check=True)
            bias01(net, 0, ps0, start=False)
            dummies(gaps.get(f"l0_{net}", 0))
        evac_relu("mu", 0, ps0, h1, EV0, (nc.vector, nc.scalar))
        evac_relu("lv", 0, ps0, h1, EV0, (nc.scalar, nc.vector))

        # L1: bias first (ready early), then DR g-outer
        for net in ("mu", "lv"):
            dummies(gaps.get(f"l1_{net}", 0))
            bias01(net, 1, ps1, start=True)
            for g in range(2):
                for mt in range(4):
                    mm(ps1[net][:, mt, :], w1[net][mt][g],
                       h1[net][:, 2 * g:2 * g + 2, :],
                       start=False, stop=(mt == 3 and g == 1), perf_mode=DR,
                       skip_group_check=True)
            engs = (nc.vector, nc.scalar) if net == "mu" else (nc.scalar, nc.vector)
            evac_relu(net, 1, ps1, h2, EV1, engs)

        # L2 row-major: psum[i, d] += sum_k h2[k, i] * W2[k, d]
        dummies(gaps.get("l2", 0))
        for net in ("mu", "lv"):
            mm(ps2[net], ones_row, brow(net, 2, m=2 * P),
               start=True, stop=False, skip_group_check=True)
        for net in ("mu", "lv"):
            dummies(gaps.get(f"l2_{net}", 0))
            for g in range(2):
                mm(ps2[net], h2[net][:, 2 * g:2 * g + 2, :], w2[net][g],
                   start=False, stop=(g == 1), perf_mode=DR,
                   skip_group_check=True)

        # stats on own b shard -> ps_out cols 4..7
        b2 = pool.tile([P, 2 * P], BF16, tag="b2")
        nc.gpsimd.tensor_mul(b2, b_v, b_v)
        for dh in range(2):
            mm(ps_out[:, 4 + dh:5 + dh], b_v[:, dh * P:(dh + 1) * P], ones_col,
               start=False, stop=False, skip_group_check=True)
        for dh in range(2):
            mm(ps_out[:, 6 + dh:7 + dh], b2[:, dh * P:(dh + 1) * P], ones_col,
               start=False, stop=False, skip_group_check=True)

        # ---- tail (row-major [i, d]) ----
        y = pool.tile([P, 2 * P], BF16, tag="y")
        nc.vector.tensor_scalar_mul(y, ps2["mu"], EV2)
        y2 = pool.tile([P, 2 * P], BF16, tag="y2")
        nsq = pool.tile([P, 1], F32, tag="nsq")
        nc.vector.scalar_tensor_tensor(y2, y, 1.0, y, op0=MUL, op1=MUL,
                                       accum_out=nsq)

        lv_sb = pool.tile([P, 2 * P], F32, tag="lv_sb")
        sd = pool.tile([P, 1], F32, tag="sd")
        nc.scalar.activation(lv_sb, ps2["lv"], AF.Tanh, scale=EV2, accum_out=sd)
        iv = pool.tile([P, 2 * P], BF16, tag="iv")
        nc.scalar.activation(iv, lv_sb, AF.Exp, scale=-1.0)

        # Newton rsqrt on DVE: seed from the int32 magic, then iterations
        rinv = pool.tile([P, 1], F32, tag="rinv")
        ri = rinv.bitcast(I32)
        nc.vector.tensor_scalar(ri, nsq.bitcast(I32), 1, None, op0=SHR)
        nc.vector.tensor_scalar(ri, ri, -1, 0x5F3759DF, op0=MUL, op1=ADD)
        nh = pool.tile([P, 1], F32, tag="nh")
        nc.vector.tensor_scalar_mul(nh, nsq, -0.5)
        t1 = pool.tile([P, 1], F32, tag="t1")
        for _ in range(opts["newton"]):
            nc.vector.tensor_mul(t1, rinv, rinv)
            nc.vector.tensor_scalar(t1, t1, nh, 1.5, op0=MUL, op1=ADD)
            nc.vector.tensor_mul(rinv, rinv, t1)

        muiv = pool.tile([P, 2 * P], BF16, tag="muiv")
        nc.vector.scalar_tensor_tensor(muiv, y, rinv, iv, op0=MUL, op1=MUL)
        y2iv = pool.tile([P, 2 * P], BF16, tag="y2iv")
        sc = pool.tile([P, 1], F32, tag="sc")
        nc.vector.scalar_tensor_tensor(y2iv, y2, 1.0, iv, op0=MUL, op1=MUL,
                                       accum_out=sc)
        nc.vector.tensor_scalar(t1, sc, rinv, None, op0=MUL)
        row = pool.tile([P, 1], F32, tag="row")
        nc.vector.scalar_tensor_tensor(row, t1, rinv, sd, op0=MUL, op1=ADD)

        # ---- SB / SC / scalar -> ps_out cols 0..3, 8 ----
        for dh in range(2):
            mm(ps_out[:, 0 + dh:1 + dh], iv[:, dh * P:(dh + 1) * P], ones_col,
               start=False, stop=False, skip_group_check=True)
        for dh in range(2):
            mm(ps_out[:, 2 + dh:3 + dh], muiv[:, dh * P:(dh + 1) * P], ones_col,
               start=False, stop=False, skip_group_check=True)
        # broadcast the scalar total to every partition of col 8 so the bank
        # is fully written (lhsT = all-ones [128, 128])
        mm(ps_out[:, 8:9], ones_sq, row,
           start=False, stop=True, skip_group_check=True)

        out_sb = pool.tile([P, 9], F32, tag="out_sb")
        nc.vector.tensor_copy(out_sb, ps_out[:, 0:9])
        nc.sync.dma_start(dram["out"][:, :], out_sb)


_NC_CACHE = {}
_OPTS = {"warmup": 20, "newton": 1}


def _build(reps=1):
    key = ("v2", reps, repr(sorted(_OPTS.items())))
    if key in _NC_CACHE:
        return _NC_CACHE[key]
    nc = bacc.Bacc("TRN2", target_bir_lowering=False, debug=False)
    dram = {
        "pack": nc.dram_tensor("pack", [P, PACK_COLS], U8, kind="ExternalInput"),
        "bias": nc.dram_tensor("bias", [1, 5120], U8, kind="ExternalInput"),
        "out": nc.dram_tensor("out", [P, 9], F32, kind="ExternalOutput"),
    }
    with tile.TileContext(nc) as tc:
        _emit(nc, tc, dram, opts=_OPTS)
    nc.compile()
    _NC_CACHE[key] = nc
    return nc


def _pack_host(inputs):
    """Build the weight/bias packs (shared across cores) and per-core packs."""
    f32 = np.float32

    def fp8(x):
        return np.asarray(x, f32).astype(NP_F8)

    w0cols = np.empty((P, W1_OFF - W0_OFF - 512), np.uint8)
    col = 0
    for net in ("mu", "lv"):
        w = fp8(np.asarray(inputs[f"{net}_w0"], f32) * S_W)  # [256, 512]
        t = w.reshape(2, P, 4, P).transpose(2, 1, 0, 3)       # [mt, p, kt, m]
        w0cols[:, col:col + 1024] = t.transpose(1, 0, 2, 3).reshape(P, 1024).view(np.uint8)
        col += 1024
    assert col == w0cols.shape[1]

    wcols = np.empty((P, PACK_COLS - W1_OFF), np.uint8)  # L1 + L2
    col = 0
    for net in ("mu", "lv"):
        w = fp8(np.asarray(inputs[f"{net}_w1"], f32) * S_W)  # [512, 512]
        # tile (mt, g): [p, kt, m] = w[(2g+kt)*128+p, mt*128+m]
        t = w.reshape(2, 2, P, 4, P)                          # [g, kt, p, mt, m]
        t = t.transpose(3, 0, 2, 1, 4)                        # [mt, g, p, kt, m]
        wcols[:, col:col + 2048] = t.transpose(2, 0, 1, 3, 4).reshape(P, 2048).view(np.uint8)
        col += 2048
    for net in ("mu", "lv"):
        w = fp8(np.asarray(inputs[f"{net}_w2"], f32) * S_W)  # [512, 256]
        t = w.reshape(2, 2, P, 2 * P)                         # [g, kt, p, m]
        t = t.transpose(0, 2, 1, 3)                           # [g, p, kt, m]
        wcols[:, col:col + 1024] = t.transpose(1, 0, 2, 3).reshape(P, 1024).view(np.uint8)
        col += 1024
    assert col == wcols.shape[1]

    brow = np.empty((1, 2560), NP_BF16)
    for net in ("mu", "lv"):
        for l, s in ((0, S_A * S_W), (1, S_H * S_W), (2, S_H * S_W)):
            off = BIAS_OFF[(net, l)]
            bb = np.asarray(inputs[f"{net}_b{l}"], f32) * s
            brow[0, off:off + bb.size] = bb.astype(NP_BF16)
    bias_u8 = np.ascontiguousarray(brow.view(np.uint8))       # [1, 5120]

    a = np.asarray(inputs["domain_a"], f32)
    b = np.asarray(inputs["domain_b"], f32)
    packs = []
    for c in range(NCORES):
        pk = np.empty((P, PACK_COLS), np.uint8)
        ash = fp8(a[c * ROWS:(c + 1) * ROWS] * S_A)           # [128 rows, 256 d]
        # a_pack[p, kt, n] = a[n, kt*128+p]
        at = ash.T.reshape(2, P, P).transpose(1, 0, 2).reshape(P, 256)
        pk[:, A_OFF:A_OFF + 256] = np.ascontiguousarray(at).view(np.uint8)
        pk[:, W0_OFF:B_OFF] = w0cols
        bsh = np.asarray(b[c * ROWS:(c + 1) * ROWS], f32).astype(NP_BF16)
        pk[:, B_OFF:W1_OFF] = np.ascontiguousarray(bsh).view(np.uint8)
        pk[:, W1_OFF:] = wcols
        packs.append(pk)
    return packs, bias_u8


def kernel_with_results(**inputs):
    import os
    try:
        import antenv.axon_hooks  # noqa: F401
    except ImportError:
        # run_bass_kernel_spmd's trace path needs this module; without it a
        # stray BASS_TRACE=1 in the environment would crash the run.
        os.environ.setdefault("BASS_NEVER_TRACE", "1")
    nc = _build()
    packs, bias_u8 = _pack_host(inputs)
    in_maps = [dict(pack=packs[c], bias=bias_u8) for c in range(NCORES)]
    res = run_bass_kernel_spmd(nc, in_maps, core_ids=list(range(NCORES)))

    scal = np.float64(0.0)
    SB = np.zeros(D, np.float64)
    SC = np.zeros(D, np.float64)
    pmb = np.zeros(D, np.float64)
    pmsq = np.zeros(D, np.float64)
    for r in res.results:
        o = np.asarray(r["out"], np.float64)
        SB += np.concatenate([o[:, 0], o[:, 1]])
        SC += np.concatenate([o[:, 2], o[:, 3]])
        pmb += np.concatenate([o[:, 4], o[:, 5]])
        pmsq += np.concatenate([o[:, 6], o[:, 7]])
        scal += o[0, 8]
    mb = pmb / N
    msq = pmsq / N
    loss = (scal + msq @ SB - 2.0 * (mb @ SC)) / N
    return np.asarray(loss, dtype=np.float32).reshape(()), res


def kernel(**inputs):
    out, _ = kernel_with_results(**inputs)
    return out
